# revision 13
# baseline (speedup 1.0000x reference)
"""Trainium2 Bass kernel for DirectConv2D (3x3 VALID, NCHW/OIHW).

Problem: x [32, 256, 56, 56] int32 (values 0..7 after clip),
         weight [256, 256, 3, 3] fp32 (small non-negative ints 0..6)
         -> out [32, 256, 54, 54] fp32.

Strategy (mode "wino8", the default):
 - Data-parallel across 8 NeuronCores: 4 images per core, weight replicated.
 - 1D Winograd F(2,3) along W cuts tensor-engine work by 1/3: per output
   row-pair column j, the four transform planes
       V0 = e_j - e_{j+1}   V1 = o_j + e_{j+1}
       V2 = e_{j+1} - o_j   V3 = o_j - o_{j+1}
   (e/o = even/odd input columns) are precomputed on the HOST and shipped
   as fp8 (all values are ints in [-7, 14] -> exact in e4m3). Transformed
   weights u0 = g0, u1 = (g0+g1+g2)/2, u2 = (g0-g1+g2)/2, u3 = g2 are
   half-integers <= 9, exact in e4m3 except sum==17 (0.9% of entries,
   rounded half up / half down -> ~1e-4 final rel err).
 - Per 18-row output tile and oc-half: 4 PSUM banks accumulate
   M_m = sum_{c,kh} u_m(kh) . V_m(r+kh) via 12 fp8 DoubleRow matmuls
   (contraction 256 = 2 c-chunks x 128, moving [128,2,18,27]).
 - Vector engine fuses the inverse transform with PSUM evacuation:
   y_even = M0 + M1 + M2, y_odd = M1 - M2 - M3, written straight into a
   dense [128, 54*54] staging tile with stride-2 column APs.
 - 288 matmuls x 205 ns vs 432 for direct conv.

Mode "fp8dr" is the previous direct-conv kernel (9 shifted matmuls per
tile); "winob"/"bf16" are structurally identical bf16 builds for CoreSim.
"""

import sys

sys.path.insert(0, "/opt/trn_rl_repo")

import ml_dtypes
import numpy as np

N_CORES = 8
IMGS = 4  # images per core
H = W = 56
OH = OW = 54
PIX = H * W  # 3136
PIXP = PIX + 4  # direct-conv padding (mode fp8dr)
ROWS_PER_TILE = 9
N_TILE = ROWS_PER_TILE * W  # 504
N_ROWTILES = OH // ROWS_PER_TILE  # 6

# Winograd layout constants
WJ = 27  # transform-domain row width (27 cols -> 54 output columns)
WROWS = 56  # input rows
VROW = WROWS * WJ  # 1568 elements per (m, c) plane per image
JQ = 27  # transform columns per row
WT_ROWS = 18  # output rows per tile
NW = WT_ROWS * JQ  # 486 outputs per matmul (<=512 fp32 PSUM bank)
N_WTILES = OH // WT_ROWS  # 3

_PROGRAM_CACHE = {}


def _build_wino(mode="wino8"):
    import concourse.bacc as bacc
    import concourse.mybir as mybir
    import concourse.tile as tile

    nc = bacc.Bacc(
        "TRN2",
        target_bir_lowering=False,
        debug=False,
        enable_asserts=False,
        num_devices=N_CORES,
    )
    dt8 = mybir.dt.float8e4
    dtb = mybir.dt.bfloat16
    dt_in = dt8 if mode == "wino8" else dtb
    fp32 = mybir.dt.float32
    alu = mybir.AluOpType

    # lead_sb: weights + image-0 V packed dense in consumption order so the
    # lead-in DMAs move multi-KB contiguous lines (small-line transfers only
    # reach ~50-100 GB/s per ring; dense ones ~300 GB/s). Per partition:
    #   [0:3072)      w oc0 [12 um][2 c][128]
    #   [3072:6144)   w oc1
    #   [6144:10464)  G0: V0 rows 0..19  [4 m][2 c][540]
    #   [10464:15216) G1: V0 rows 18..39 [4 m][2 c][594]
    #   [15216:19536) G2: V0 rows 36..55 [4 m][2 c][540]
    LW = 6144
    G0, G1, G2 = LW, LW + 4320, LW + 4320 + 4752
    LEAD = G2 + 4320
    lead_d = nc.dram_tensor("lead_sb", [128, LEAD], dt_in, kind="ExternalInput").ap()
    # v_sb: [128 ki, 3 img (1..3), 4 m, 2 c, 1512 (56 rows x 27 cols)]
    v_d = nc.dram_tensor("v_sb", [128, IMGS - 1, 4, 2, VROW], dt_in, kind="ExternalInput").ap()
    out_d = nc.dram_tensor(
        "out", [IMGS, 256, OH, OW], fp32, kind="ExternalOutput"
    ).ap()

    V0M_BASE = 18 * WJ  # G1 holds rows 18..39
    V0Z_BASE = 36 * WJ  # G2 holds rows 36..55

    with tile.TileContext(nc) as tc:
        with (
            tc.tile_pool(name="const", bufs=1) as const_pool,
            tc.tile_pool(name="psum", bufs=8, space="PSUM") as psum_pool,
            tc.tile_pool(name="scr", bufs=8) as scr_pool,
            tc.tile_pool(name="outs", bufs=4) as out_pool,
        ):
            # PE warm-up on scratch while the lead input DMAs are in flight.
            w_warm = const_pool.tile([128, 2, 128], dt_in)
            x_warm = const_pool.tile([128, 2, 544], dt_in)
            if mode != "wino8":
                nc.gpsimd.memset(w_warm, 0.0)
                nc.gpsimd.memset(x_warm, 0.0)
            else:
                nc.gpsimd.memset(w_warm[:, 0, 0:2], 0.0)
                nc.gpsimd.memset(x_warm[:, 0, 0:2], 0.0)
            pt_warm = psum_pool.tile([128, NW], fp32, tag="pt")
            N_WARM = 9
            for i in range(N_WARM):
                rhs_w = x_warm[:, :, 0:NW]
                if mode == "wino8":
                    nc.tensor.matmul(
                        pt_warm, w_warm, rhs_w,
                        start=(i == 0), stop=(i == N_WARM - 1),
                        perf_mode=mybir.MatmulPerfMode.DoubleRow,
                    )
                else:
                    nc.tensor.matmul(
                        pt_warm, w_warm[:, 0], rhs_w[:, 0],
                        start=(i == 0), stop=(i == N_WARM - 1),
                    )

            lead_t = const_pool.tile([128, LEAD], dt_in)
            wt = lead_t[:, 0:LW].rearrange(
                "p (a b c d) -> p a b c d", a=2, b=12, c=2, d=128
            )
            vt0a = lead_t[:, G0:G1].rearrange("p (m c v) -> p m c v", m=4, v=540)
            vt0m = lead_t[:, G1:G2].rearrange("p (m c v) -> p m c v", m=4, v=594)
            vt0z = lead_t[:, G2:LEAD].rearrange("p (m c v) -> p m c v", m=4, v=540)
            vts = [None] + [
                const_pool.tile([128, 4, 2, VROW], dt_in, name=f"vt{n}", tag=f"vt{n}")
                for n in (1, 2, 3)
            ]
            # Lead-in: few dense transfers, ordered by first use, split across
            # both rings; image 1..3 V planes as m-pair chunks (6KB lines).
            def lchunk(eng, a, b):
                eng.dma_start(out=lead_t[:, a:b], in_=lead_d[:, a:b])

            # sync ring:
            lchunk(nc.sync, 0, 1536)            # w oc0 m0,m1
            lchunk(nc.sync, G0 + 2160, G1)      # V0 rows 0-19 m2,m3
            lchunk(nc.sync, 3072, 4608)         # w oc1 m0,m1
            lchunk(nc.sync, G1, G1 + 2376)      # V0 rows 18-39 m0,m1
            nc.sync.dma_start(out=vts[1][:, 0:2], in_=v_d[:, 0, 0:2])
            nc.sync.dma_start(out=vts[3][:, 0:2], in_=v_d[:, 2, 0:2])
            nc.sync.dma_start(out=vts[2][:, 2:4], in_=v_d[:, 1, 2:4])
            # scalar ring:
            lchunk(nc.scalar, G0, G0 + 2160)    # V0 rows 0-19 m0,m1
            lchunk(nc.scalar, 1536, 3072)       # w oc0 m2,m3
            lchunk(nc.scalar, 4608, 6144)       # w oc1 m2,m3
            lchunk(nc.scalar, G1 + 2376, G2)    # V0 rows 18-39 m2,m3
            lchunk(nc.scalar, G2, LEAD)         # V0 rows 36-55
            nc.scalar.dma_start(out=vts[1][:, 2:4], in_=v_d[:, 0, 2:4])
            nc.scalar.dma_start(out=vts[2][:, 0:2], in_=v_d[:, 1, 0:2])
            nc.scalar.dma_start(out=vts[3][:, 2:4], in_=v_d[:, 2, 2:4])

            def v_src(n, t):
                """(V tile, element base) holding rows needed by row tile t."""
                if n == 0:
                    if t == 0:
                        return vt0a, 0
                    if t == 1:
                        return vt0m, V0M_BASE
                    return vt0z, V0Z_BASE
                return vts[n], 0

            for n in range(IMGS):
                ots = [out_pool.tile([128, OH * OW], fp32, name="ot", tag="ot")
                       for _ in range(2)]
                for t in range(N_WTILES):
                    r0 = t * WT_ROWS
                    vsrc, vbase = v_src(n, t)
                    for oc in range(2):
                        last_tile = n == IMGS - 1 and oc == 1 and t == N_WTILES - 1
                        pts = [
                            psum_pool.tile([128, NW], fp32, name="pt", tag="pt")
                            for m in range(4)
                        ]
                        for m in range(4):
                            for kh in range(3):
                                off = (r0 + kh) * WJ - vbase
                                if mode == "wino8":
                                    rhs = vsrc[:, m, :, off : off + NW]
                                    nc.tensor.matmul(
                                        pts[m],
                                        wt[:, oc, m * 3 + kh],
                                        rhs,
                                        start=(kh == 0),
                                        stop=(kh == 2),
                                        perf_mode=mybir.MatmulPerfMode.DoubleRow,
                                    )
                                else:
                                    for c in range(2):
                                        rhs = vsrc[:, m, c, off : off + NW]
                                        nc.tensor.matmul(
                                            pts[m],
                                            wt[:, oc, m * 3 + kh, c],
                                            rhs,
                                            start=(kh == 0 and c == 0),
                                            stop=(kh == 2 and c == 1),
                                        )
                        # inverse transform fused with PSUM evacuation, split
                        # over three engines so no single one paces the PE:
                        #   ACT:    a = M1, b = M2   (PSUM reads)
                        #   GpSimd: S = a + b, D = a - b  (SBUF only)
                        #   DVE:    y_even = M0 + S, y_odd = -M3 + D
                        base = r0 * OW
                        blk = ots[oc][:, base : base + WT_ROWS * OW].rearrange(
                            "p (r j e) -> p e r j", j=JQ, e=2
                        )
                        p3 = [pts[m].rearrange("p (r q) -> p r q", q=JQ)
                              for m in range(4)]
                        aa = scr_pool.tile([128, WT_ROWS, JQ], fp32, name="scr", tag="scr")
                        bb = scr_pool.tile([128, WT_ROWS, JQ], fp32, name="scr", tag="scr")
                        ss = scr_pool.tile([128, WT_ROWS, JQ], fp32, name="scr", tag="scr")
                        dd = scr_pool.tile([128, WT_ROWS, JQ], fp32, name="scr", tag="scr")
                        if last_tile:
                            halves = [(0, 5), (5, 9), (9, 14), (14, 18)]
                        else:
                            halves = [(0, WT_ROWS)]
                        for hi, (ra, rb) in enumerate(halves):
                            sl = slice(ra, rb)
                            nc.scalar.copy(out=aa[:, sl], in_=p3[1][:, sl])
                            nc.scalar.copy(out=bb[:, sl], in_=p3[2][:, sl])
                            nc.gpsimd.tensor_add(ss[:, sl], aa[:, sl], bb[:, sl])
                            nc.vector.scalar_tensor_tensor(
                                out=dd[:, sl], in0=bb[:, sl], scalar=-1.0,
                                in1=aa[:, sl], op0=alu.mult, op1=alu.add,
                            )
                            nc.vector.scalar_tensor_tensor(
                                out=blk[:, 0, sl], in0=p3[0][:, sl], scalar=0.0,
                                in1=ss[:, sl], op0=alu.bypass, op1=alu.add,
                            )
                            nc.vector.scalar_tensor_tensor(
                                out=blk[:, 1, sl], in0=p3[3][:, sl], scalar=-1.0,
                                in1=dd[:, sl], op0=alu.mult, op1=alu.add,
                            )
                            if n == IMGS - 1:
                                # last image: store per (t, oc) slice so the
                                # trailing store stays small; alternate rings.
                                ra2, rb2 = r0 + ra, r0 + rb
                                eng = nc.sync if (t + oc + hi) % 2 == 0 else nc.scalar
                                eng.dma_start(
                                    out=out_d[n, oc * 128 : (oc + 1) * 128, ra2:rb2, :],
                                    in_=ots[oc][:, ra2 * OW : rb2 * OW].rearrange(
                                        "p (h w) -> p h w", w=OW
                                    ),
                                )
                if n < IMGS - 1:
                    for oc in range(2):
                        eng = nc.sync if oc == 0 else nc.scalar
                        eng.dma_start(
                            out=out_d[n, oc * 128 : (oc + 1) * 128, :, :],
                            in_=ots[oc].rearrange("p (h w) -> p h w", w=OW),
                        )
    nc.compile()
    return nc


def _build_direct(mode="fp8dr"):
    import concourse.bacc as bacc
    import concourse.mybir as mybir
    import concourse.tile as tile

    nc = bacc.Bacc(
        "TRN2",
        target_bir_lowering=False,
        debug=False,
        enable_asserts=False,
        num_devices=N_CORES,
    )
    dt8 = mybir.dt.float8e4
    dtb = mybir.dt.bfloat16
    dt_in = dt8 if mode == "fp8dr" else dtb

    x_d = nc.dram_tensor("x_sb", [128, 2, IMGS, PIXP], dt_in, kind="ExternalInput").ap()
    w_d = nc.dram_tensor("w_sb", [128, 2, 9, 2, 128], dt_in, kind="ExternalInput").ap()
    out_d = nc.dram_tensor(
        "out", [IMGS, 256, OH, OW], mybir.dt.float32, kind="ExternalOutput"
    ).ap()

    NT486 = ROWS_PER_TILE * OW  # 486
    X0A_END = 1232
    X0M_BASE, X0M_END = 1008, 2140
    X0Z_BASE = 2016

    with tile.TileContext(nc) as tc:
        with (
            tc.tile_pool(name="const", bufs=1) as const_pool,
            tc.tile_pool(name="psum", bufs=8, space="PSUM") as psum_pool,
            tc.tile_pool(name="outs", bufs=3) as out_pool,
        ):
            w_warm = const_pool.tile([128, 2, 128], dt_in)
            x_warm = const_pool.tile([128, 2, 544], dt_in)
            if mode != "fp8dr":
                nc.gpsimd.memset(w_warm, 0.0)
                nc.gpsimd.memset(x_warm, 0.0)
            else:
                nc.gpsimd.memset(w_warm[:, 0, 0:2], 0.0)
                nc.gpsimd.memset(x_warm[:, 0, 0:2], 0.0)
            pt_warm = psum_pool.tile([128, NT486], mybir.dt.float32, tag="pt")
            N_WARM = 13
            for i in range(N_WARM):
                rhs_w = x_warm[:, :, 0:N_TILE].rearrange(
                    "p c (r q) -> p c r q", q=W
                )[:, :, :, 0:OW]
                if mode == "fp8dr":
                    nc.tensor.matmul(
                        pt_warm, w_warm, rhs_w,
                        start=(i == 0), stop=(i == N_WARM - 1),
                        perf_mode=mybir.MatmulPerfMode.DoubleRow,
                    )
                else:
                    nc.tensor.matmul(
                        pt_warm, w_warm[:, 0], rhs_w[:, 0],
                        start=(i == 0), stop=(i == N_WARM - 1),
                    )

            wt = const_pool.tile([128, 2, 9, 2, 128], dt_in)
            xt0a = const_pool.tile([128, 2, X0A_END], dt_in)
            xt0m = const_pool.tile([128, 2, X0M_END - X0M_BASE], dt_in)
            xt0z = const_pool.tile([128, 2, PIXP - X0Z_BASE], dt_in)
            xts = [None] + [
                const_pool.tile([128, 2, PIXP], dt_in, name=f"xt{n}", tag=f"xt{n}")
                for n in (1, 2, 3)
            ]
            nc.sync.dma_start(out=wt[:, 0, 0], in_=w_d[:, 0, 0])
            nc.sync.dma_start(out=xt0a[:, 0, 0:620], in_=x_d[:, 0, 0, 0:620])
            nc.sync.dma_start(out=wt[:, 0, 1:], in_=w_d[:, 0, 1:])
            nc.sync.dma_start(out=wt[:, 1], in_=w_d[:, 1])
            for c in range(2):
                nc.sync.dma_start(out=xts[1][:, c], in_=x_d[:, c, 1])
            nc.scalar.dma_start(out=xt0a[:, 1, 0:620], in_=x_d[:, 1, 0, 0:620])
            for c in range(2):
                nc.scalar.dma_start(
                    out=xt0a[:, c, 620:], in_=x_d[:, c, 0, 620:X0A_END]
                )
            for c in range(2):
                nc.scalar.dma_start(
                    out=xt0m[:, c], in_=x_d[:, c, 0, X0M_BASE:X0M_END]
                )
            for c in range(2):
                nc.scalar.dma_start(out=xt0z[:, c], in_=x_d[:, c, 0, X0Z_BASE:])
            for n in (2, 3):
                for c in range(2):
                    nc.scalar.dma_start(out=xts[n][:, c], in_=x_d[:, c, n])

            def x_src(n, t):
                if n == 0:
                    if t < 2:
                        return xt0a, 0
                    if t < 4:
                        return xt0m, X0M_BASE
                    return xt0z, X0Z_BASE
                return xts[n], 0

            for n in range(IMGS):
                for oc in range(2):
                    ot = out_pool.tile([128, OH * OW], mybir.dt.float32)
                    for t in range(N_ROWTILES):
                        h0 = t * ROWS_PER_TILE
                        xsrc, xbase = x_src(n, t)
                        pt = psum_pool.tile([128, NT486], mybir.dt.float32)
                        k = 0
                        for kh in range(3):
                            for kw in range(3):
                                off = (h0 + kh) * W + kw - xbase
                                if mode == "fp8dr":
                                    rhs = xsrc[:, :, off : off + N_TILE].rearrange(
                                        "p c (r q) -> p c r q", q=W
                                    )[:, :, :, 0:OW]
                                    nc.tensor.matmul(
                                        pt,
                                        wt[:, oc, k, :, :],
                                        rhs,
                                        start=(k == 0),
                                        stop=(k == 8),
                                        perf_mode=mybir.MatmulPerfMode.DoubleRow,
                                    )
                                else:
                                    for c in range(2):
                                        rhs = xsrc[:, c, off : off + N_TILE].rearrange(
                                            "p (r q) -> p r q", q=W
                                        )[:, :, 0:OW]
                                        nc.tensor.matmul(
                                            pt,
                                            wt[:, oc, k, c, :],
                                            rhs,
                                            start=(k == 0 and c == 0),
                                            stop=(k == 8 and c == 1),
                                        )
                                k += 1
                        last_block = n == IMGS - 1 and oc == 1
                        if last_block and t == N_ROWTILES - 1:
                            s = 5 * OW
                            base = t * NT486
                            nc.vector.tensor_copy(
                                out=ot[:, base : base + s], in_=pt[:, 0:s]
                            )
                            nc.sync.dma_start(
                                out=out_d[n, oc * 128 : (oc + 1) * 128,
                                          h0 : h0 + 5, :],
                                in_=ot[:, base : base + s].rearrange(
                                    "p (h w) -> p h w", w=OW
                                ),
                            )
                            nc.vector.tensor_copy(
                                out=ot[:, base + s : base + NT486],
                                in_=pt[:, s:NT486],
                            )
                            nc.scalar.dma_start(
                                out=out_d[n, oc * 128 : (oc + 1) * 128,
                                          h0 + 5 : h0 + ROWS_PER_TILE, :],
                                in_=ot[:, base + s : base + NT486].rearrange(
                                    "p (h w) -> p h w", w=OW
                                ),
                            )
                        else:
                            nc.vector.tensor_copy(
                                out=ot[:, t * NT486 : (t + 1) * NT486], in_=pt
                            )
                        if last_block:
                            if t in (1, 3):
                                nc.sync.dma_start(
                                    out=out_d[n, oc * 128 : (oc + 1) * 128,
                                              h0 - ROWS_PER_TILE : h0 + ROWS_PER_TILE, :],
                                    in_=ot[:, (t - 1) * NT486 : (t + 1) * NT486].rearrange(
                                        "p (h w) -> p h w", w=OW
                                    ),
                                )
                            elif t == 4:
                                nc.sync.dma_start(
                                    out=out_d[n, oc * 128 : (oc + 1) * 128,
                                              h0 : h0 + ROWS_PER_TILE, :],
                                    in_=ot[:, t * NT486 : (t + 1) * NT486].rearrange(
                                        "p (h w) -> p h w", w=OW
                                    ),
                                )
                    if not last_block:
                        nc.sync.dma_start(
                            out=out_d[n, oc * 128 : (oc + 1) * 128, :, :],
                            in_=ot.rearrange("p (h w) -> p h w", w=OW),
                        )
    nc.compile()
    return nc


def _build_program(mode):
    if mode in ("wino8", "winob"):
        return _build_wino(mode)
    return _build_direct(mode)


def get_program(mode="wino8"):
    if mode not in _PROGRAM_CACHE:
        _PROGRAM_CACHE[mode] = _build_program(mode)
    return _PROGRAM_CACHE[mode]


def _np_dtype(mode):
    return ml_dtypes.float8_e4m3 if mode in ("fp8dr", "wino8") else ml_dtypes.bfloat16


def prep_weight_wino(weight, mode="wino8"):
    """[256, 256, 3, 3] OIHW -> w_sb [128 ki, 2 oc, 12 (m*3+kh), 2 c, 128 m]."""
    wq = weight.astype(np.int32)
    wq = wq.reshape(2, 128, 2, 128, 3, 3)  # [ocb, m, c, ki, kh, kw]
    g0 = wq[..., 0].astype(np.float64)
    g1 = wq[..., 1].astype(np.float64)
    g2 = wq[..., 2].astype(np.float64)
    u = np.stack([g0, (g0 + g1 + g2) / 2, (g0 - g1 + g2) / 2, g2], axis=0)
    # [u, ocb, m, c, ki, kh]
    if mode == "wino8":
        # 17/2 = 8.5 is not representable in e4m3; round half of the
        # occurrences up and half down so the error is ~zero-mean.
        s = (g0 + g1 + g2).astype(np.int64)
        idx = np.indices(s.shape).sum(axis=0)  # checkerboard parity
        u[1] = np.where(s == 17, np.where(idx % 2 == 0, 8.0, 9.0), u[1])
    w_sb = u.transpose(4, 1, 0, 5, 3, 2)  # [ki, ocb, u, kh, c, m]
    w_sb = np.ascontiguousarray(w_sb).reshape(128, 2, 12, 2, 128)
    return w_sb.astype(_np_dtype(mode))


def prep_x_wino(x_core, mode="wino8"):
    """x_core [IMGS, 256, 56, 56] int32 -> v_sb [128 ki, IMGS, 4 m, 2 c, 1568]."""
    xq = np.clip(x_core.astype(np.int32), 0, 7)
    e = xq[..., 0::2]  # [n, ch, 56, 28]
    o = xq[..., 1::2]
    e0, e1 = e[..., :JQ], e[..., 1:]
    o0, o1 = o[..., :JQ], o[..., 1:]
    v = np.empty((IMGS, 256, 4, WROWS, WJ), np.float32)
    v[:, :, 0] = e0 - e1
    v[:, :, 1] = o0 + e1
    v[:, :, 2] = e1 - o0
    v[:, :, 3] = o0 - o1
    v = v.reshape(IMGS, 2, 128, 4, VROW)  # [n, c, ki, m, row]
    v_sb = np.ascontiguousarray(v.transpose(2, 0, 3, 1, 4))  # [ki, n, m, c, row]
    return v_sb.astype(_np_dtype(mode))


def prep_weight(weight, mode="fp8dr"):
    """weight [256, 256, 3, 3] OIHW fp32 -> w_sb [128 ki, 2 oc, 9 tap, 2 c, 128 m]."""
    wq = weight.astype(np.int32).astype(np.float32)
    wq = wq.reshape(2, 128, 2, 128, 3, 3)
    w_sb = np.ascontiguousarray(wq.transpose(3, 0, 4, 5, 2, 1))
    w_sb = w_sb.reshape(128, 2, 9, 2, 128)
    return w_sb.astype(_np_dtype(mode))


def prep_x_core(x_core, mode="fp8dr"):
    """x_core [IMGS, 256, 56, 56] int32 -> x_sb [128 ki, 2 c, IMGS, PIXP]."""
    xq = np.clip(x_core.astype(np.int32), 0, 7).astype(np.float32)
    xq = xq.reshape(IMGS, 2, 128, PIX)
    x_sb = np.zeros((128, 2, IMGS, PIXP), np.float32)
    x_sb[:, :, :, :PIX] = xq.transpose(2, 1, 0, 3)
    return x_sb.astype(_np_dtype(mode))


def make_lead(w_sb, v_sb):
    """Pack weights + image-0 V into the dense lead tensor."""
    lead = np.empty((128, 6144 + 4320 + 4752 + 4320), w_sb.dtype)
    lead[:, 0:6144] = w_sb.reshape(128, -1)
    lead[:, 6144:10464] = v_sb[:, 0, :, :, 0:540].reshape(128, -1)
    lead[:, 10464:15216] = v_sb[:, 0, :, :, 486:1080].reshape(128, -1)
    lead[:, 15216:19536] = v_sb[:, 0, :, :, 972:1512].reshape(128, -1)
    return lead


def make_in_maps(x, weight, mode="wino8"):
    if mode in ("wino8", "winob"):
        w_sb = prep_weight_wino(weight, mode)
        maps = []
        for c in range(N_CORES):
            v_sb = prep_x_wino(x[c * IMGS : (c + 1) * IMGS], mode)
            maps.append({
                "lead_sb": make_lead(w_sb, v_sb),
                "v_sb": np.ascontiguousarray(v_sb[:, 1:]),
            })
        return maps
    w_sb = prep_weight(weight, mode)
    return [
        {"x_sb": prep_x_core(x[c * IMGS : (c + 1) * IMGS], mode), "w_sb": w_sb}
        for c in range(N_CORES)
    ]


def kernel(x, weight):
    import time

    from concourse.bass_utils import run_bass_kernel_spmd

    mode = "wino8"
    nc = get_program(mode)
    in_maps = make_in_maps(np.asarray(x), np.asarray(weight), mode)
    last_err = None
    for attempt in range(3):
        try:
            res = run_bass_kernel_spmd(nc, in_maps, list(range(N_CORES)))
            break
        except Exception as e:  # transient NRT_EXEC_UNIT_UNRECOVERABLE flakes
            last_err = e
            time.sleep(2.0)
    else:
        raise last_err
    return np.concatenate(
        [res.results[c]["out"] for c in range(N_CORES)], axis=0
    ).astype(np.float32)


# revision 20
# speedup vs baseline: 1.1427x; 1.1427x over previous
"""Trainium2 Bass kernel for DirectConv2D (3x3 VALID, NCHW/OIHW).

Problem: x [32, 256, 56, 56] int32 (values 0..7 after clip),
         weight [256, 256, 3, 3] fp32 (small non-negative ints 0..6)
         -> out [32, 256, 54, 54] fp32.

Strategy (mode "wino8", the default):
 - Data-parallel across 8 NeuronCores: 4 images per core, weight replicated.
 - 1D Winograd F(2,3) along W cuts tensor-engine work by 1/3: per output
   row-pair column j, the four transform planes
       V0 = e_j - e_{j+1}   V1 = o_j + e_{j+1}
       V2 = e_{j+1} - o_j   V3 = o_j - o_{j+1}
   (e/o = even/odd input columns) are precomputed on the HOST and shipped
   as fp8 (all values are ints in [-7, 14] -> exact in e4m3). Transformed
   weights u0 = g0, u1 = (g0+g1+g2)/2, u2 = (g0-g1+g2)/2, u3 = g2 are
   half-integers <= 9, exact in e4m3 except sum==17 (0.9% of entries,
   rounded half up / half down -> ~1e-4 final rel err).
 - Per 18-row output tile and oc-half: 4 PSUM banks accumulate
   M_m = sum_{c,kh} u_m(kh) . V_m(r+kh) via 12 fp8 DoubleRow matmuls
   (contraction 256 = 2 c-chunks x 128, moving [128, 2, 486] flat since
   the 18 rows x 27 cols are contiguous at stride 27).
 - The inverse transform (y_even = M0+M1+M2, y_odd = M1-M2-M3) is fused
   with PSUM evacuation and split over three engines so none paces the
   PE: ACT copies M1/M2 out of PSUM, GpSimd adds them, DVE does the two
   PSUM-reading combines, writing straight into a dense [128, 54*54]
   staging tile with stride-2 column APs.
 - Weights + image-0 V ship in one densely packed "lead" DRAM tensor so
   the lead-in DMAs move multi-KB contiguous lines (~300 GB/s/ring vs
   ~50-100 for sub-KB lines); 9 warm-up matmuls bridge the DMA wait so
   the HAM clock-gate opens (K=8/8) right as the real stream starts.
 - 288 matmuls x 205 ns vs 432 for direct conv: ~81.5 us vs 107.6 us.

Mode "fp8dr" is the previous direct-conv kernel (9 shifted matmuls per
tile); "winob"/"bf16" are structurally identical bf16 builds for CoreSim.
"""

import sys

sys.path.insert(0, "/opt/trn_rl_repo")

import ml_dtypes
import numpy as np

N_CORES = 8
IMGS = 4  # images per core
H = W = 56
OH = OW = 54
PIX = H * W  # 3136
PIXP = PIX + 4  # direct-conv padding (mode fp8dr)
ROWS_PER_TILE = 9
N_TILE = ROWS_PER_TILE * W  # 504
N_ROWTILES = OH // ROWS_PER_TILE  # 6

# Winograd layout constants
WJ = 27  # transform-domain row width (27 cols -> 54 output columns)
WROWS = 56  # input rows
VROW = WROWS * WJ  # 1568 elements per (m, c) plane per image
JQ = 27  # transform columns per row
WT_ROWS = 18  # output rows per tile
NW = WT_ROWS * JQ  # 486 outputs per matmul (<=512 fp32 PSUM bank)
N_WTILES = OH // WT_ROWS  # 3

_PROGRAM_CACHE = {}


def _build_wino(mode="wino8"):
    import concourse.bacc as bacc
    import concourse.mybir as mybir
    import concourse.tile as tile

    nc = bacc.Bacc(
        "TRN2",
        target_bir_lowering=False,
        debug=False,
        enable_asserts=False,
        num_devices=N_CORES,
    )
    dt8 = mybir.dt.float8e4
    dtb = mybir.dt.bfloat16
    dt_in = dt8 if mode == "wino8" else dtb
    fp32 = mybir.dt.float32
    alu = mybir.AluOpType

    # lead_sb: weights + image-0 V packed dense in consumption order so the
    # lead-in DMAs move multi-KB contiguous lines (small-line transfers only
    # reach ~50-100 GB/s per ring; dense ones ~300 GB/s). Per partition:
    #   [0:3072)      w oc0 [12 um][2 c][128]
    #   [3072:6144)   w oc1
    #   [6144:10464)  G0: V0 rows 0..19  [4 m][2 c][540]
    #   [10464:15216) G1: V0 rows 18..39 [4 m][2 c][594]
    #   [15216:19536) G2: V0 rows 36..55 [4 m][2 c][540]
    LW = 6144
    G0, G1, G2 = LW, LW + 4320, LW + 4320 + 4752
    LEAD = G2 + 4320
    lead_d = nc.dram_tensor("lead_sb", [128, LEAD], dt_in, kind="ExternalInput").ap()
    # v_sb: [128 ki, 3 img (1..3), 4 m, 2 c, 1512 (56 rows x 27 cols)]
    v_d = nc.dram_tensor("v_sb", [128, IMGS - 1, 4, 2, VROW], dt_in, kind="ExternalInput").ap()
    out_d = nc.dram_tensor(
        "out", [IMGS, 256, OH, OW], fp32, kind="ExternalOutput"
    ).ap()

    V0M_BASE = 18 * WJ  # G1 holds rows 18..39
    V0Z_BASE = 36 * WJ  # G2 holds rows 36..55

    with tile.TileContext(nc) as tc:
        with (
            tc.tile_pool(name="const", bufs=1) as const_pool,
            tc.tile_pool(name="psum", bufs=8, space="PSUM") as psum_pool,
            tc.tile_pool(name="scr", bufs=16) as scr_pool,
            tc.tile_pool(name="outs", bufs=4) as out_pool,
        ):
            # PE warm-up on scratch while the lead input DMAs are in flight.
            w_warm = const_pool.tile([128, 2, 128], dt_in)
            x_warm = const_pool.tile([128, 2, 544], dt_in)
            if mode != "wino8":
                nc.gpsimd.memset(w_warm, 0.0)
                nc.gpsimd.memset(x_warm, 0.0)
            else:
                nc.gpsimd.memset(w_warm[:, 0, 0:2], 0.0)
                nc.gpsimd.memset(x_warm[:, 0, 0:2], 0.0)
            pt_warm = psum_pool.tile([128, NW], fp32, tag="pt")
            N_WARM = 9
            for i in range(N_WARM):
                rhs_w = x_warm[:, :, 0:NW]
                if mode == "wino8":
                    nc.tensor.matmul(
                        pt_warm, w_warm, rhs_w,
                        start=(i == 0), stop=(i == N_WARM - 1),
                        perf_mode=mybir.MatmulPerfMode.DoubleRow,
                    )
                else:
                    nc.tensor.matmul(
                        pt_warm, w_warm[:, 0], rhs_w[:, 0],
                        start=(i == 0), stop=(i == N_WARM - 1),
                    )

            lead_t = const_pool.tile([128, LEAD], dt_in)
            wt = lead_t[:, 0:LW].rearrange(
                "p (a b c d) -> p a b c d", a=2, b=12, c=2, d=128
            )
            vt0a = lead_t[:, G0:G1].rearrange("p (m c v) -> p m c v", m=4, v=540)
            vt0m = lead_t[:, G1:G2].rearrange("p (m c v) -> p m c v", m=4, v=594)
            vt0z = lead_t[:, G2:LEAD].rearrange("p (m c v) -> p m c v", m=4, v=540)
            vts = [None] + [
                const_pool.tile([128, 4, 2, VROW], dt_in, name=f"vt{n}", tag=f"vt{n}")
                for n in (1, 2, 3)
            ]
            # Lead-in: few dense transfers, ordered by first use, split across
            # both rings; image 1..3 V planes as m-pair chunks (6KB lines).
            def lchunk(eng, a, b):
                eng.dma_start(out=lead_t[:, a:b], in_=lead_d[:, a:b])

            # sync ring:
            lchunk(nc.sync, 0, 1536)            # w oc0 m0,m1
            lchunk(nc.sync, G0 + 2160, G1)      # V0 rows 0-19 m2,m3
            lchunk(nc.sync, 3072, 4608)         # w oc1 m0,m1
            lchunk(nc.sync, G1, G1 + 2376)      # V0 rows 18-39 m0,m1
            nc.sync.dma_start(out=vts[1][:, 0:2], in_=v_d[:, 0, 0:2])
            nc.sync.dma_start(out=vts[3][:, 0:2], in_=v_d[:, 2, 0:2])
            nc.sync.dma_start(out=vts[2][:, 2:4], in_=v_d[:, 1, 2:4])
            # scalar ring:
            lchunk(nc.scalar, G0, G0 + 2160)    # V0 rows 0-19 m0,m1
            lchunk(nc.scalar, 1536, 3072)       # w oc0 m2,m3
            lchunk(nc.scalar, 4608, 6144)       # w oc1 m2,m3
            lchunk(nc.scalar, G1 + 2376, G2)    # V0 rows 18-39 m2,m3
            lchunk(nc.scalar, G2, LEAD)         # V0 rows 36-55
            nc.scalar.dma_start(out=vts[1][:, 2:4], in_=v_d[:, 0, 2:4])
            nc.scalar.dma_start(out=vts[2][:, 0:2], in_=v_d[:, 1, 0:2])
            nc.scalar.dma_start(out=vts[3][:, 2:4], in_=v_d[:, 2, 2:4])

            def v_src(n, t):
                """(V tile, element base) holding rows needed by row tile t."""
                if n == 0:
                    if t == 0:
                        return vt0a, 0
                    if t == 1:
                        return vt0m, V0M_BASE
                    return vt0z, V0Z_BASE
                return vts[n], 0

            for n in range(IMGS):
                ots = [out_pool.tile([128, OH * OW], fp32, name="ot", tag="ot")
                       for _ in range(2)]
                for t in range(N_WTILES):
                    r0 = t * WT_ROWS
                    vsrc, vbase = v_src(n, t)
                    for oc in range(2):
                        last_tile = n == IMGS - 1 and oc == 1 and t == N_WTILES - 1
                        pts = [
                            psum_pool.tile([128, NW], fp32, name="pt", tag="pt")
                            for m in range(4)
                        ]
                        for m in range(4):
                            for kh in range(3):
                                off = (r0 + kh) * WJ - vbase
                                if mode == "wino8":
                                    rhs = vsrc[:, m, :, off : off + NW]
                                    nc.tensor.matmul(
                                        pts[m],
                                        wt[:, oc, m * 3 + kh],
                                        rhs,
                                        start=(kh == 0),
                                        stop=(kh == 2),
                                        perf_mode=mybir.MatmulPerfMode.DoubleRow,
                                    )
                                else:
                                    for c in range(2):
                                        rhs = vsrc[:, m, c, off : off + NW]
                                        nc.tensor.matmul(
                                            pts[m],
                                            wt[:, oc, m * 3 + kh, c],
                                            rhs,
                                            start=(kh == 0 and c == 0),
                                            stop=(kh == 2 and c == 1),
                                        )
                        # inverse transform fused with PSUM evacuation, split
                        # over three engines so no single one paces the PE:
                        #   ACT:    a = M1, b = M2   (PSUM reads)
                        #   GpSimd: S = a + b, D = a - b  (SBUF only)
                        #   DVE:    y_even = M0 + S, y_odd = -M3 + D
                        base = r0 * OW
                        blk = ots[oc][:, base : base + WT_ROWS * OW].rearrange(
                            "p (r j e) -> p e r j", j=JQ, e=2
                        )
                        p3 = [pts[m].rearrange("p (r q) -> p r q", q=JQ)
                              for m in range(4)]
                        aa = scr_pool.tile([128, WT_ROWS, JQ], fp32, name="scr", tag="scr")
                        bb = scr_pool.tile([128, WT_ROWS, JQ], fp32, name="scr", tag="scr")
                        ss = scr_pool.tile([128, WT_ROWS, JQ], fp32, name="scr", tag="scr")
                        dd = scr_pool.tile([128, WT_ROWS, JQ], fp32, name="scr", tag="scr")
                        if last_tile:
                            halves = [(0, 5), (5, 9), (9, 14), (14, 18)]
                        else:
                            halves = [(0, WT_ROWS)]
                        for hi, (ra, rb) in enumerate(halves):
                            sl = slice(ra, rb)
                            nc.scalar.copy(out=aa[:, sl], in_=p3[1][:, sl])
                            nc.scalar.copy(out=bb[:, sl], in_=p3[2][:, sl])
                            nc.gpsimd.tensor_add(ss[:, sl], aa[:, sl], bb[:, sl])
                            nc.vector.scalar_tensor_tensor(
                                out=dd[:, sl], in0=bb[:, sl], scalar=-1.0,
                                in1=aa[:, sl], op0=alu.mult, op1=alu.add,
                            )
                            nc.vector.scalar_tensor_tensor(
                                out=blk[:, 0, sl], in0=p3[0][:, sl], scalar=0.0,
                                in1=ss[:, sl], op0=alu.bypass, op1=alu.add,
                            )
                            nc.vector.scalar_tensor_tensor(
                                out=blk[:, 1, sl], in0=p3[3][:, sl], scalar=-1.0,
                                in1=dd[:, sl], op0=alu.mult, op1=alu.add,
                            )
                            if n == IMGS - 1:
                                # last image: store per (t, oc) slice so the
                                # trailing store stays small; alternate rings.
                                ra2, rb2 = r0 + ra, r0 + rb
                                eng = nc.sync if (t + oc + hi) % 2 == 0 else nc.scalar
                                eng.dma_start(
                                    out=out_d[n, oc * 128 : (oc + 1) * 128, ra2:rb2, :],
                                    in_=ots[oc][:, ra2 * OW : rb2 * OW].rearrange(
                                        "p (h w) -> p h w", w=OW
                                    ),
                                )
                if n < IMGS - 1:
                    for oc in range(2):
                        eng = nc.sync if oc == 0 else nc.scalar
                        eng.dma_start(
                            out=out_d[n, oc * 128 : (oc + 1) * 128, :, :],
                            in_=ots[oc].rearrange("p (h w) -> p h w", w=OW),
                        )
    nc.compile()
    return nc


def _build_direct(mode="fp8dr"):
    import concourse.bacc as bacc
    import concourse.mybir as mybir
    import concourse.tile as tile

    nc = bacc.Bacc(
        "TRN2",
        target_bir_lowering=False,
        debug=False,
        enable_asserts=False,
        num_devices=N_CORES,
    )
    dt8 = mybir.dt.float8e4
    dtb = mybir.dt.bfloat16
    dt_in = dt8 if mode == "fp8dr" else dtb

    x_d = nc.dram_tensor("x_sb", [128, 2, IMGS, PIXP], dt_in, kind="ExternalInput").ap()
    w_d = nc.dram_tensor("w_sb", [128, 2, 9, 2, 128], dt_in, kind="ExternalInput").ap()
    out_d = nc.dram_tensor(
        "out", [IMGS, 256, OH, OW], mybir.dt.float32, kind="ExternalOutput"
    ).ap()

    NT486 = ROWS_PER_TILE * OW  # 486
    X0A_END = 1232
    X0M_BASE, X0M_END = 1008, 2140
    X0Z_BASE = 2016

    with tile.TileContext(nc) as tc:
        with (
            tc.tile_pool(name="const", bufs=1) as const_pool,
            tc.tile_pool(name="psum", bufs=8, space="PSUM") as psum_pool,
            tc.tile_pool(name="outs", bufs=3) as out_pool,
        ):
            w_warm = const_pool.tile([128, 2, 128], dt_in)
            x_warm = const_pool.tile([128, 2, 544], dt_in)
            if mode != "fp8dr":
                nc.gpsimd.memset(w_warm, 0.0)
                nc.gpsimd.memset(x_warm, 0.0)
            else:
                nc.gpsimd.memset(w_warm[:, 0, 0:2], 0.0)
                nc.gpsimd.memset(x_warm[:, 0, 0:2], 0.0)
            pt_warm = psum_pool.tile([128, NT486], mybir.dt.float32, tag="pt")
            N_WARM = 13
            for i in range(N_WARM):
                rhs_w = x_warm[:, :, 0:N_TILE].rearrange(
                    "p c (r q) -> p c r q", q=W
                )[:, :, :, 0:OW]
                if mode == "fp8dr":
                    nc.tensor.matmul(
                        pt_warm, w_warm, rhs_w,
                        start=(i == 0), stop=(i == N_WARM - 1),
                        perf_mode=mybir.MatmulPerfMode.DoubleRow,
                    )
                else:
                    nc.tensor.matmul(
                        pt_warm, w_warm[:, 0], rhs_w[:, 0],
                        start=(i == 0), stop=(i == N_WARM - 1),
                    )

            wt = const_pool.tile([128, 2, 9, 2, 128], dt_in)
            xt0a = const_pool.tile([128, 2, X0A_END], dt_in)
            xt0m = const_pool.tile([128, 2, X0M_END - X0M_BASE], dt_in)
            xt0z = const_pool.tile([128, 2, PIXP - X0Z_BASE], dt_in)
            xts = [None] + [
                const_pool.tile([128, 2, PIXP], dt_in, name=f"xt{n}", tag=f"xt{n}")
                for n in (1, 2, 3)
            ]
            nc.sync.dma_start(out=wt[:, 0, 0], in_=w_d[:, 0, 0])
            nc.sync.dma_start(out=xt0a[:, 0, 0:620], in_=x_d[:, 0, 0, 0:620])
            nc.sync.dma_start(out=wt[:, 0, 1:], in_=w_d[:, 0, 1:])
            nc.sync.dma_start(out=wt[:, 1], in_=w_d[:, 1])
            for c in range(2):
                nc.sync.dma_start(out=xts[1][:, c], in_=x_d[:, c, 1])
            nc.scalar.dma_start(out=xt0a[:, 1, 0:620], in_=x_d[:, 1, 0, 0:620])
            for c in range(2):
                nc.scalar.dma_start(
                    out=xt0a[:, c, 620:], in_=x_d[:, c, 0, 620:X0A_END]
                )
            for c in range(2):
                nc.scalar.dma_start(
                    out=xt0m[:, c], in_=x_d[:, c, 0, X0M_BASE:X0M_END]
                )
            for c in range(2):
                nc.scalar.dma_start(out=xt0z[:, c], in_=x_d[:, c, 0, X0Z_BASE:])
            for n in (2, 3):
                for c in range(2):
                    nc.scalar.dma_start(out=xts[n][:, c], in_=x_d[:, c, n])

            def x_src(n, t):
                if n == 0:
                    if t < 2:
                        return xt0a, 0
                    if t < 4:
                        return xt0m, X0M_BASE
                    return xt0z, X0Z_BASE
                return xts[n], 0

            for n in range(IMGS):
                for oc in range(2):
                    ot = out_pool.tile([128, OH * OW], mybir.dt.float32)
                    for t in range(N_ROWTILES):
                        h0 = t * ROWS_PER_TILE
                        xsrc, xbase = x_src(n, t)
                        pt = psum_pool.tile([128, NT486], mybir.dt.float32)
                        k = 0
                        for kh in range(3):
                            for kw in range(3):
                                off = (h0 + kh) * W + kw - xbase
                                if mode == "fp8dr":
                                    rhs = xsrc[:, :, off : off + N_TILE].rearrange(
                                        "p c (r q) -> p c r q", q=W
                                    )[:, :, :, 0:OW]
                                    nc.tensor.matmul(
                                        pt,
                                        wt[:, oc, k, :, :],
                                        rhs,
                                        start=(k == 0),
                                        stop=(k == 8),
                                        perf_mode=mybir.MatmulPerfMode.DoubleRow,
                                    )
                                else:
                                    for c in range(2):
                                        rhs = xsrc[:, c, off : off + N_TILE].rearrange(
                                            "p (r q) -> p r q", q=W
                                        )[:, :, 0:OW]
                                        nc.tensor.matmul(
                                            pt,
                                            wt[:, oc, k, c, :],
                                            rhs,
                                            start=(k == 0 and c == 0),
                                            stop=(k == 8 and c == 1),
                                        )
                                k += 1
                        last_block = n == IMGS - 1 and oc == 1
                        if last_block and t == N_ROWTILES - 1:
                            s = 5 * OW
                            base = t * NT486
                            nc.vector.tensor_copy(
                                out=ot[:, base : base + s], in_=pt[:, 0:s]
                            )
                            nc.sync.dma_start(
                                out=out_d[n, oc * 128 : (oc + 1) * 128,
                                          h0 : h0 + 5, :],
                                in_=ot[:, base : base + s].rearrange(
                                    "p (h w) -> p h w", w=OW
                                ),
                            )
                            nc.vector.tensor_copy(
                                out=ot[:, base + s : base + NT486],
                                in_=pt[:, s:NT486],
                            )
                            nc.scalar.dma_start(
                                out=out_d[n, oc * 128 : (oc + 1) * 128,
                                          h0 + 5 : h0 + ROWS_PER_TILE, :],
                                in_=ot[:, base + s : base + NT486].rearrange(
                                    "p (h w) -> p h w", w=OW
                                ),
                            )
                        else:
                            nc.vector.tensor_copy(
                                out=ot[:, t * NT486 : (t + 1) * NT486], in_=pt
                            )
                        if last_block:
                            if t in (1, 3):
                                nc.sync.dma_start(
                                    out=out_d[n, oc * 128 : (oc + 1) * 128,
                                              h0 - ROWS_PER_TILE : h0 + ROWS_PER_TILE, :],
                                    in_=ot[:, (t - 1) * NT486 : (t + 1) * NT486].rearrange(
                                        "p (h w) -> p h w", w=OW
                                    ),
                                )
                            elif t == 4:
                                nc.sync.dma_start(
                                    out=out_d[n, oc * 128 : (oc + 1) * 128,
                                              h0 : h0 + ROWS_PER_TILE, :],
                                    in_=ot[:, t * NT486 : (t + 1) * NT486].rearrange(
                                        "p (h w) -> p h w", w=OW
                                    ),
                                )
                    if not last_block:
                        nc.sync.dma_start(
                            out=out_d[n, oc * 128 : (oc + 1) * 128, :, :],
                            in_=ot.rearrange("p (h w) -> p h w", w=OW),
                        )
    nc.compile()
    return nc


def _build_program(mode):
    if mode in ("wino8", "winob"):
        return _build_wino(mode)
    return _build_direct(mode)


def get_program(mode="wino8"):
    if mode not in _PROGRAM_CACHE:
        _PROGRAM_CACHE[mode] = _build_program(mode)
    return _PROGRAM_CACHE[mode]


def _np_dtype(mode):
    return ml_dtypes.float8_e4m3 if mode in ("fp8dr", "wino8") else ml_dtypes.bfloat16


def prep_weight_wino(weight, mode="wino8"):
    """[256, 256, 3, 3] OIHW -> w_sb [128 ki, 2 oc, 12 (m*3+kh), 2 c, 128 m]."""
    wq = weight.astype(np.int32)
    wq = wq.reshape(2, 128, 2, 128, 3, 3)  # [ocb, m, c, ki, kh, kw]
    g0 = wq[..., 0].astype(np.float64)
    g1 = wq[..., 1].astype(np.float64)
    g2 = wq[..., 2].astype(np.float64)
    u = np.stack([g0, (g0 + g1 + g2) / 2, (g0 - g1 + g2) / 2, g2], axis=0)
    # [u, ocb, m, c, ki, kh]
    if mode == "wino8":
        # 17/2 = 8.5 is not representable in e4m3; round half of the
        # occurrences up and half down so the error is ~zero-mean.
        s = (g0 + g1 + g2).astype(np.int64)
        idx = np.indices(s.shape).sum(axis=0)  # checkerboard parity
        u[1] = np.where(s == 17, np.where(idx % 2 == 0, 8.0, 9.0), u[1])
    w_sb = u.transpose(4, 1, 0, 5, 3, 2)  # [ki, ocb, u, kh, c, m]
    w_sb = np.ascontiguousarray(w_sb).reshape(128, 2, 12, 2, 128)
    return w_sb.astype(_np_dtype(mode))


def prep_x_wino(x_core, mode="wino8"):
    """x_core [IMGS, 256, 56, 56] int32 -> v_sb [128 ki, IMGS, 4 m, 2 c, 1568]."""
    xq = np.clip(x_core.astype(np.int32), 0, 7)
    e = xq[..., 0::2]  # [n, ch, 56, 28]
    o = xq[..., 1::2]
    e0, e1 = e[..., :JQ], e[..., 1:]
    o0, o1 = o[..., :JQ], o[..., 1:]
    v = np.empty((IMGS, 256, 4, WROWS, WJ), np.float32)
    v[:, :, 0] = e0 - e1
    v[:, :, 1] = o0 + e1
    v[:, :, 2] = e1 - o0
    v[:, :, 3] = o0 - o1
    v = v.reshape(IMGS, 2, 128, 4, VROW)  # [n, c, ki, m, row]
    v_sb = np.ascontiguousarray(v.transpose(2, 0, 3, 1, 4))  # [ki, n, m, c, row]
    return v_sb.astype(_np_dtype(mode))


def prep_weight(weight, mode="fp8dr"):
    """weight [256, 256, 3, 3] OIHW fp32 -> w_sb [128 ki, 2 oc, 9 tap, 2 c, 128 m]."""
    wq = weight.astype(np.int32).astype(np.float32)
    wq = wq.reshape(2, 128, 2, 128, 3, 3)
    w_sb = np.ascontiguousarray(wq.transpose(3, 0, 4, 5, 2, 1))
    w_sb = w_sb.reshape(128, 2, 9, 2, 128)
    return w_sb.astype(_np_dtype(mode))


def prep_x_core(x_core, mode="fp8dr"):
    """x_core [IMGS, 256, 56, 56] int32 -> x_sb [128 ki, 2 c, IMGS, PIXP]."""
    xq = np.clip(x_core.astype(np.int32), 0, 7).astype(np.float32)
    xq = xq.reshape(IMGS, 2, 128, PIX)
    x_sb = np.zeros((128, 2, IMGS, PIXP), np.float32)
    x_sb[:, :, :, :PIX] = xq.transpose(2, 1, 0, 3)
    return x_sb.astype(_np_dtype(mode))


def make_lead(w_sb, v_sb):
    """Pack weights + image-0 V into the dense lead tensor."""
    lead = np.empty((128, 6144 + 4320 + 4752 + 4320), w_sb.dtype)
    lead[:, 0:6144] = w_sb.reshape(128, -1)
    lead[:, 6144:10464] = v_sb[:, 0, :, :, 0:540].reshape(128, -1)
    lead[:, 10464:15216] = v_sb[:, 0, :, :, 486:1080].reshape(128, -1)
    lead[:, 15216:19536] = v_sb[:, 0, :, :, 972:1512].reshape(128, -1)
    return lead


def make_in_maps(x, weight, mode="wino8"):
    if mode in ("wino8", "winob"):
        w_sb = prep_weight_wino(weight, mode)
        maps = []
        for c in range(N_CORES):
            v_sb = prep_x_wino(x[c * IMGS : (c + 1) * IMGS], mode)
            maps.append({
                "lead_sb": make_lead(w_sb, v_sb),
                "v_sb": np.ascontiguousarray(v_sb[:, 1:]),
            })
        return maps
    w_sb = prep_weight(weight, mode)
    return [
        {"x_sb": prep_x_core(x[c * IMGS : (c + 1) * IMGS], mode), "w_sb": w_sb}
        for c in range(N_CORES)
    ]


def kernel(x, weight):
    import time

    from concourse.bass_utils import run_bass_kernel_spmd

    mode = "wino8"
    nc = get_program(mode)
    in_maps = make_in_maps(np.asarray(x), np.asarray(weight), mode)
    last_err = None
    for attempt in range(3):
        try:
            res = run_bass_kernel_spmd(nc, in_maps, list(range(N_CORES)))
            break
        except Exception as e:  # transient NRT_EXEC_UNIT_UNRECOVERABLE flakes
            last_err = e
            time.sleep(2.0)
    else:
        raise last_err
    return np.concatenate(
        [res.results[c]["out"] for c in range(N_CORES)], axis=0
    ).astype(np.float32)


# revision 21
# speedup vs baseline: 1.1446x; 1.0016x over previous
"""Trainium2 Bass kernel for DirectConv2D (3x3 VALID, NCHW/OIHW).

Problem: x [32, 256, 56, 56] int32 (values 0..7 after clip),
         weight [256, 256, 3, 3] fp32 (small non-negative ints 0..6)
         -> out [32, 256, 54, 54] fp32.

Strategy (mode "wino8", the default):
 - Data-parallel across 8 NeuronCores: 4 images per core, weight replicated.
 - 1D Winograd F(2,3) along W cuts tensor-engine work by 1/3: per output
   row-pair column j, the four transform planes
       V0 = e_j - e_{j+1}   V1 = o_j + e_{j+1}
       V2 = e_{j+1} - o_j   V3 = o_j - o_{j+1}
   (e/o = even/odd input columns) are precomputed on the HOST and shipped
   as fp8 (all values are ints in [-7, 14] -> exact in e4m3). Transformed
   weights u0 = g0, u1 = (g0+g1+g2)/2, u2 = (g0-g1+g2)/2, u3 = g2 are
   half-integers <= 9, exact in e4m3 except sum==17 (0.9% of entries,
   rounded half up / half down -> ~1e-4 final rel err).
 - Per 18-row output tile and oc-half: 4 PSUM banks accumulate
   M_m = sum_{c,kh} u_m(kh) . V_m(r+kh) via 12 fp8 DoubleRow matmuls
   (contraction 256 = 2 c-chunks x 128, moving [128, 2, 486] flat since
   the 18 rows x 27 cols are contiguous at stride 27).
 - The inverse transform (y_even = M0+M1+M2, y_odd = M1-M2-M3) is fused
   with PSUM evacuation and split over three engines so none paces the
   PE: ACT copies M1/M2 out of PSUM, GpSimd adds them, DVE does the two
   PSUM-reading combines, writing straight into a dense [128, 54*54]
   staging tile with stride-2 column APs.
 - Weights + image-0 V ship in one densely packed "lead" DRAM tensor so
   the lead-in DMAs move multi-KB contiguous lines (~300 GB/s/ring vs
   ~50-100 for sub-KB lines); 9 warm-up matmuls bridge the DMA wait so
   the HAM clock-gate opens (K=8/8) right as the real stream starts.
 - 288 matmuls x 205 ns vs 432 for direct conv: ~81.5 us vs 107.6 us.

Mode "fp8dr" is the previous direct-conv kernel (9 shifted matmuls per
tile); "winob"/"bf16" are structurally identical bf16 builds for CoreSim.
"""

import sys

sys.path.insert(0, "/opt/trn_rl_repo")

import ml_dtypes
import numpy as np

N_CORES = 8
IMGS = 4  # images per core
H = W = 56
OH = OW = 54
PIX = H * W  # 3136
PIXP = PIX + 4  # direct-conv padding (mode fp8dr)
ROWS_PER_TILE = 9
N_TILE = ROWS_PER_TILE * W  # 504
N_ROWTILES = OH // ROWS_PER_TILE  # 6

# Winograd layout constants
WJ = 27  # transform-domain row width (27 cols -> 54 output columns)
WROWS = 56  # input rows
VROW = WROWS * WJ  # 1568 elements per (m, c) plane per image
JQ = 27  # transform columns per row
WT_ROWS = 18  # output rows per tile
NW = WT_ROWS * JQ  # 486 outputs per matmul (<=512 fp32 PSUM bank)
N_WTILES = OH // WT_ROWS  # 3

_PROGRAM_CACHE = {}


def _build_wino(mode="wino8"):
    import concourse.bacc as bacc
    import concourse.mybir as mybir
    import concourse.tile as tile

    nc = bacc.Bacc(
        "TRN2",
        target_bir_lowering=False,
        debug=False,
        enable_asserts=False,
        num_devices=N_CORES,
    )
    dt8 = mybir.dt.float8e4
    dtb = mybir.dt.bfloat16
    dt_in = dt8 if mode == "wino8" else dtb
    fp32 = mybir.dt.float32
    alu = mybir.AluOpType

    # lead_sb: weights + image-0 V packed dense in consumption order so the
    # lead-in DMAs move multi-KB contiguous lines (small-line transfers only
    # reach ~50-100 GB/s per ring; dense ones ~300 GB/s). Per partition:
    #   [0:3072)      w oc0 [12 um][2 c][128]
    #   [3072:6144)   w oc1
    #   [6144:10464)  G0: V0 rows 0..19  [4 m][2 c][540]
    #   [10464:15216) G1: V0 rows 18..39 [4 m][2 c][594]
    #   [15216:19536) G2: V0 rows 36..55 [4 m][2 c][540]
    LW = 6144
    G0, G1, G2 = LW, LW + 4320, LW + 4320 + 4752
    LEAD = G2 + 4320
    lead_d = nc.dram_tensor("lead_sb", [128, LEAD], dt_in, kind="ExternalInput").ap()
    # v_sb: [128 ki, 3 img (1..3), 4 m, 2 c, 1512 (56 rows x 27 cols)]
    v_d = nc.dram_tensor("v_sb", [128, IMGS - 1, 4, 2, VROW], dt_in, kind="ExternalInput").ap()
    out_d = nc.dram_tensor(
        "out", [IMGS, 256, OH, OW], fp32, kind="ExternalOutput"
    ).ap()

    V0M_BASE = 18 * WJ  # G1 holds rows 18..39
    V0Z_BASE = 36 * WJ  # G2 holds rows 36..55

    with tile.TileContext(nc) as tc:
        with (
            tc.tile_pool(name="const", bufs=1) as const_pool,
            tc.tile_pool(name="psum", bufs=8, space="PSUM") as psum_pool,
            tc.tile_pool(name="scr", bufs=16) as scr_pool,
            tc.tile_pool(name="outs", bufs=4) as out_pool,
        ):
            # PE warm-up on scratch while the lead input DMAs are in flight.
            w_warm = const_pool.tile([128, 2, 128], dt_in)
            x_warm = const_pool.tile([128, 2, 544], dt_in)
            if mode != "wino8":
                nc.gpsimd.memset(w_warm, 0.0)
                nc.gpsimd.memset(x_warm, 0.0)
            else:
                nc.gpsimd.memset(w_warm[:, 0, 0:2], 0.0)
                nc.gpsimd.memset(x_warm[:, 0, 0:2], 0.0)
            pt_warm = psum_pool.tile([128, NW], fp32, tag="pt")
            N_WARM = 9
            for i in range(N_WARM):
                rhs_w = x_warm[:, :, 0:NW]
                if mode == "wino8":
                    nc.tensor.matmul(
                        pt_warm, w_warm, rhs_w,
                        start=(i == 0), stop=(i == N_WARM - 1),
                        perf_mode=mybir.MatmulPerfMode.DoubleRow,
                    )
                else:
                    nc.tensor.matmul(
                        pt_warm, w_warm[:, 0], rhs_w[:, 0],
                        start=(i == 0), stop=(i == N_WARM - 1),
                    )

            lead_t = const_pool.tile([128, LEAD], dt_in)
            wt = lead_t[:, 0:LW].rearrange(
                "p (a b c d) -> p a b c d", a=2, b=12, c=2, d=128
            )
            vt0a = lead_t[:, G0:G1].rearrange("p (m c v) -> p m c v", m=4, v=540)
            vt0m = lead_t[:, G1:G2].rearrange("p (m c v) -> p m c v", m=4, v=594)
            vt0z = lead_t[:, G2:LEAD].rearrange("p (m c v) -> p m c v", m=4, v=540)
            vts = [None] + [
                const_pool.tile([128, 4, 2, VROW], dt_in, name=f"vt{n}", tag=f"vt{n}")
                for n in (1, 2, 3)
            ]
            # Lead-in: few dense transfers, ordered by first use, split across
            # both rings; image 1..3 V planes as m-pair chunks (6KB lines).
            def lchunk(eng, a, b):
                eng.dma_start(out=lead_t[:, a:b], in_=lead_d[:, a:b])

            # sync ring:
            lchunk(nc.sync, 0, 1536)            # w oc0 m0,m1
            lchunk(nc.sync, G0 + 2160, G1)      # V0 rows 0-19 m2,m3
            lchunk(nc.sync, 3072, 4608)         # w oc1 m0,m1
            lchunk(nc.sync, G1, G1 + 2376)      # V0 rows 18-39 m0,m1
            nc.sync.dma_start(out=vts[1][:, 0:2], in_=v_d[:, 0, 0:2])
            nc.sync.dma_start(out=vts[3][:, 0:2], in_=v_d[:, 2, 0:2])
            nc.sync.dma_start(out=vts[2][:, 2:4], in_=v_d[:, 1, 2:4])
            # scalar ring:
            lchunk(nc.scalar, G0, G0 + 2160)    # V0 rows 0-19 m0,m1
            lchunk(nc.scalar, 1536, 3072)       # w oc0 m2,m3
            lchunk(nc.scalar, 4608, 6144)       # w oc1 m2,m3
            lchunk(nc.scalar, G1 + 2376, G2)    # V0 rows 18-39 m2,m3
            lchunk(nc.scalar, G2, LEAD)         # V0 rows 36-55
            nc.scalar.dma_start(out=vts[1][:, 2:4], in_=v_d[:, 0, 2:4])
            nc.scalar.dma_start(out=vts[2][:, 0:2], in_=v_d[:, 1, 0:2])
            nc.scalar.dma_start(out=vts[3][:, 2:4], in_=v_d[:, 2, 2:4])

            def v_src(n, t):
                """(V tile, element base) holding rows needed by row tile t."""
                if n == 0:
                    if t == 0:
                        return vt0a, 0
                    if t == 1:
                        return vt0m, V0M_BASE
                    return vt0z, V0Z_BASE
                return vts[n], 0

            for n in range(IMGS):
                ots = [out_pool.tile([128, OH * OW], fp32, name="ot", tag="ot")
                       for _ in range(2)]
                for t in range(N_WTILES):
                    r0 = t * WT_ROWS
                    vsrc, vbase = v_src(n, t)
                    for oc in range(2):
                        last_tile = n == IMGS - 1 and oc == 1 and t == N_WTILES - 1
                        pts = [
                            psum_pool.tile([128, NW], fp32, name="pt", tag="pt")
                            for m in range(4)
                        ]
                        for m in range(4):
                            for kh in range(3):
                                off = (r0 + kh) * WJ - vbase
                                if mode == "wino8":
                                    rhs = vsrc[:, m, :, off : off + NW]
                                    nc.tensor.matmul(
                                        pts[m],
                                        wt[:, oc, m * 3 + kh],
                                        rhs,
                                        start=(kh == 0),
                                        stop=(kh == 2),
                                        perf_mode=mybir.MatmulPerfMode.DoubleRow,
                                    )
                                else:
                                    for c in range(2):
                                        rhs = vsrc[:, m, c, off : off + NW]
                                        nc.tensor.matmul(
                                            pts[m],
                                            wt[:, oc, m * 3 + kh, c],
                                            rhs,
                                            start=(kh == 0 and c == 0),
                                            stop=(kh == 2 and c == 1),
                                        )
                        # inverse transform fused with PSUM evacuation, split
                        # over three engines so no single one paces the PE:
                        #   ACT:    a = M1, b = M2   (PSUM reads)
                        #   GpSimd: S = a + b, D = a - b  (SBUF only)
                        #   DVE:    y_even = M0 + S, y_odd = -M3 + D
                        base = r0 * OW
                        blk = ots[oc][:, base : base + WT_ROWS * OW].rearrange(
                            "p (r j e) -> p e r j", j=JQ, e=2
                        )
                        p3 = [pts[m].rearrange("p (r q) -> p r q", q=JQ)
                              for m in range(4)]
                        aa = scr_pool.tile([128, WT_ROWS, JQ], fp32, name="scr", tag="scr")
                        bb = scr_pool.tile([128, WT_ROWS, JQ], fp32, name="scr", tag="scr")
                        ss = scr_pool.tile([128, WT_ROWS, JQ], fp32, name="scr", tag="scr")
                        dd = scr_pool.tile([128, WT_ROWS, JQ], fp32, name="scr", tag="scr")
                        if last_tile:
                            halves = [(0, 5), (5, 9), (9, 14), (14, 18)]
                        else:
                            halves = [(0, WT_ROWS)]
                        for hi, (ra, rb) in enumerate(halves):
                            sl = slice(ra, rb)
                            nc.scalar.copy(out=aa[:, sl], in_=p3[1][:, sl])
                            nc.scalar.copy(out=bb[:, sl], in_=p3[2][:, sl])
                            nc.gpsimd.tensor_add(ss[:, sl], aa[:, sl], bb[:, sl])
                            nc.vector.scalar_tensor_tensor(
                                out=dd[:, sl], in0=bb[:, sl], scalar=-1.0,
                                in1=aa[:, sl], op0=alu.mult, op1=alu.add,
                            )
                            nc.vector.scalar_tensor_tensor(
                                out=blk[:, 0, sl], in0=p3[0][:, sl], scalar=0.0,
                                in1=ss[:, sl], op0=alu.bypass, op1=alu.add,
                            )
                            nc.vector.scalar_tensor_tensor(
                                out=blk[:, 1, sl], in0=p3[3][:, sl], scalar=-1.0,
                                in1=dd[:, sl], op0=alu.mult, op1=alu.add,
                            )
                            if n == IMGS - 1:
                                # last image: store per (t, oc) slice so the
                                # trailing store stays small; alternate rings.
                                # The very last full tile (t2,oc0) stores as
                                # two half-row DMAs, one per ring: it issues
                                # after the MM stream ends, and halving it
                                # shortens the terminal store-queue drain.
                                ra2, rb2 = r0 + ra, r0 + rb
                                if t == N_WTILES - 1 and oc == 0:
                                    mid = (ra2 + rb2) // 2
                                    pieces = [(ra2, mid, nc.sync),
                                              (mid, rb2, nc.scalar)]
                                else:
                                    pieces = [(ra2, rb2,
                                               nc.sync if (t + oc + hi) % 2 == 0
                                               else nc.scalar)]
                                for pa, pb, eng in pieces:
                                    eng.dma_start(
                                        out=out_d[n, oc * 128 : (oc + 1) * 128, pa:pb, :],
                                        in_=ots[oc][:, pa * OW : pb * OW].rearrange(
                                            "p (h w) -> p h w", w=OW
                                        ),
                                    )
                if n < IMGS - 1:
                    for oc in range(2):
                        eng = nc.sync if oc == 0 else nc.scalar
                        eng.dma_start(
                            out=out_d[n, oc * 128 : (oc + 1) * 128, :, :],
                            in_=ots[oc].rearrange("p (h w) -> p h w", w=OW),
                        )
    nc.compile()
    return nc


def _build_direct(mode="fp8dr"):
    import concourse.bacc as bacc
    import concourse.mybir as mybir
    import concourse.tile as tile

    nc = bacc.Bacc(
        "TRN2",
        target_bir_lowering=False,
        debug=False,
        enable_asserts=False,
        num_devices=N_CORES,
    )
    dt8 = mybir.dt.float8e4
    dtb = mybir.dt.bfloat16
    dt_in = dt8 if mode == "fp8dr" else dtb

    x_d = nc.dram_tensor("x_sb", [128, 2, IMGS, PIXP], dt_in, kind="ExternalInput").ap()
    w_d = nc.dram_tensor("w_sb", [128, 2, 9, 2, 128], dt_in, kind="ExternalInput").ap()
    out_d = nc.dram_tensor(
        "out", [IMGS, 256, OH, OW], mybir.dt.float32, kind="ExternalOutput"
    ).ap()

    NT486 = ROWS_PER_TILE * OW  # 486
    X0A_END = 1232
    X0M_BASE, X0M_END = 1008, 2140
    X0Z_BASE = 2016

    with tile.TileContext(nc) as tc:
        with (
            tc.tile_pool(name="const", bufs=1) as const_pool,
            tc.tile_pool(name="psum", bufs=8, space="PSUM") as psum_pool,
            tc.tile_pool(name="outs", bufs=3) as out_pool,
        ):
            w_warm = const_pool.tile([128, 2, 128], dt_in)
            x_warm = const_pool.tile([128, 2, 544], dt_in)
            if mode != "fp8dr":
                nc.gpsimd.memset(w_warm, 0.0)
                nc.gpsimd.memset(x_warm, 0.0)
            else:
                nc.gpsimd.memset(w_warm[:, 0, 0:2], 0.0)
                nc.gpsimd.memset(x_warm[:, 0, 0:2], 0.0)
            pt_warm = psum_pool.tile([128, NT486], mybir.dt.float32, tag="pt")
            N_WARM = 13
            for i in range(N_WARM):
                rhs_w = x_warm[:, :, 0:N_TILE].rearrange(
                    "p c (r q) -> p c r q", q=W
                )[:, :, :, 0:OW]
                if mode == "fp8dr":
                    nc.tensor.matmul(
                        pt_warm, w_warm, rhs_w,
                        start=(i == 0), stop=(i == N_WARM - 1),
                        perf_mode=mybir.MatmulPerfMode.DoubleRow,
                    )
                else:
                    nc.tensor.matmul(
                        pt_warm, w_warm[:, 0], rhs_w[:, 0],
                        start=(i == 0), stop=(i == N_WARM - 1),
                    )

            wt = const_pool.tile([128, 2, 9, 2, 128], dt_in)
            xt0a = const_pool.tile([128, 2, X0A_END], dt_in)
            xt0m = const_pool.tile([128, 2, X0M_END - X0M_BASE], dt_in)
            xt0z = const_pool.tile([128, 2, PIXP - X0Z_BASE], dt_in)
            xts = [None] + [
                const_pool.tile([128, 2, PIXP], dt_in, name=f"xt{n}", tag=f"xt{n}")
                for n in (1, 2, 3)
            ]
            nc.sync.dma_start(out=wt[:, 0, 0], in_=w_d[:, 0, 0])
            nc.sync.dma_start(out=xt0a[:, 0, 0:620], in_=x_d[:, 0, 0, 0:620])
            nc.sync.dma_start(out=wt[:, 0, 1:], in_=w_d[:, 0, 1:])
            nc.sync.dma_start(out=wt[:, 1], in_=w_d[:, 1])
            for c in range(2):
                nc.sync.dma_start(out=xts[1][:, c], in_=x_d[:, c, 1])
            nc.scalar.dma_start(out=xt0a[:, 1, 0:620], in_=x_d[:, 1, 0, 0:620])
            for c in range(2):
                nc.scalar.dma_start(
                    out=xt0a[:, c, 620:], in_=x_d[:, c, 0, 620:X0A_END]
                )
            for c in range(2):
                nc.scalar.dma_start(
                    out=xt0m[:, c], in_=x_d[:, c, 0, X0M_BASE:X0M_END]
                )
            for c in range(2):
                nc.scalar.dma_start(out=xt0z[:, c], in_=x_d[:, c, 0, X0Z_BASE:])
            for n in (2, 3):
                for c in range(2):
                    nc.scalar.dma_start(out=xts[n][:, c], in_=x_d[:, c, n])

            def x_src(n, t):
                if n == 0:
                    if t < 2:
                        return xt0a, 0
                    if t < 4:
                        return xt0m, X0M_BASE
                    return xt0z, X0Z_BASE
                return xts[n], 0

            for n in range(IMGS):
                for oc in range(2):
                    ot = out_pool.tile([128, OH * OW], mybir.dt.float32)
                    for t in range(N_ROWTILES):
                        h0 = t * ROWS_PER_TILE
                        xsrc, xbase = x_src(n, t)
                        pt = psum_pool.tile([128, NT486], mybir.dt.float32)
                        k = 0
                        for kh in range(3):
                            for kw in range(3):
                                off = (h0 + kh) * W + kw - xbase
                                if mode == "fp8dr":
                                    rhs = xsrc[:, :, off : off + N_TILE].rearrange(
                                        "p c (r q) -> p c r q", q=W
                                    )[:, :, :, 0:OW]
                                    nc.tensor.matmul(
                                        pt,
                                        wt[:, oc, k, :, :],
                                        rhs,
                                        start=(k == 0),
                                        stop=(k == 8),
                                        perf_mode=mybir.MatmulPerfMode.DoubleRow,
                                    )
                                else:
                                    for c in range(2):
                                        rhs = xsrc[:, c, off : off + N_TILE].rearrange(
                                            "p (r q) -> p r q", q=W
                                        )[:, :, 0:OW]
                                        nc.tensor.matmul(
                                            pt,
                                            wt[:, oc, k, c, :],
                                            rhs,
                                            start=(k == 0 and c == 0),
                                            stop=(k == 8 and c == 1),
                                        )
                                k += 1
                        last_block = n == IMGS - 1 and oc == 1
                        if last_block and t == N_ROWTILES - 1:
                            s = 5 * OW
                            base = t * NT486
                            nc.vector.tensor_copy(
                                out=ot[:, base : base + s], in_=pt[:, 0:s]
                            )
                            nc.sync.dma_start(
                                out=out_d[n, oc * 128 : (oc + 1) * 128,
                                          h0 : h0 + 5, :],
                                in_=ot[:, base : base + s].rearrange(
                                    "p (h w) -> p h w", w=OW
                                ),
                            )
                            nc.vector.tensor_copy(
                                out=ot[:, base + s : base + NT486],
                                in_=pt[:, s:NT486],
                            )
                            nc.scalar.dma_start(
                                out=out_d[n, oc * 128 : (oc + 1) * 128,
                                          h0 + 5 : h0 + ROWS_PER_TILE, :],
                                in_=ot[:, base + s : base + NT486].rearrange(
                                    "p (h w) -> p h w", w=OW
                                ),
                            )
                        else:
                            nc.vector.tensor_copy(
                                out=ot[:, t * NT486 : (t + 1) * NT486], in_=pt
                            )
                        if last_block:
                            if t in (1, 3):
                                nc.sync.dma_start(
                                    out=out_d[n, oc * 128 : (oc + 1) * 128,
                                              h0 - ROWS_PER_TILE : h0 + ROWS_PER_TILE, :],
                                    in_=ot[:, (t - 1) * NT486 : (t + 1) * NT486].rearrange(
                                        "p (h w) -> p h w", w=OW
                                    ),
                                )
                            elif t == 4:
                                nc.sync.dma_start(
                                    out=out_d[n, oc * 128 : (oc + 1) * 128,
                                              h0 : h0 + ROWS_PER_TILE, :],
                                    in_=ot[:, t * NT486 : (t + 1) * NT486].rearrange(
                                        "p (h w) -> p h w", w=OW
                                    ),
                                )
                    if not last_block:
                        nc.sync.dma_start(
                            out=out_d[n, oc * 128 : (oc + 1) * 128, :, :],
                            in_=ot.rearrange("p (h w) -> p h w", w=OW),
                        )
    nc.compile()
    return nc


def _build_program(mode):
    if mode in ("wino8", "winob"):
        return _build_wino(mode)
    return _build_direct(mode)


def get_program(mode="wino8"):
    if mode not in _PROGRAM_CACHE:
        _PROGRAM_CACHE[mode] = _build_program(mode)
    return _PROGRAM_CACHE[mode]


def _np_dtype(mode):
    return ml_dtypes.float8_e4m3 if mode in ("fp8dr", "wino8") else ml_dtypes.bfloat16


def prep_weight_wino(weight, mode="wino8"):
    """[256, 256, 3, 3] OIHW -> w_sb [128 ki, 2 oc, 12 (m*3+kh), 2 c, 128 m]."""
    wq = weight.astype(np.int32)
    wq = wq.reshape(2, 128, 2, 128, 3, 3)  # [ocb, m, c, ki, kh, kw]
    g0 = wq[..., 0].astype(np.float64)
    g1 = wq[..., 1].astype(np.float64)
    g2 = wq[..., 2].astype(np.float64)
    u = np.stack([g0, (g0 + g1 + g2) / 2, (g0 - g1 + g2) / 2, g2], axis=0)
    # [u, ocb, m, c, ki, kh]
    if mode == "wino8":
        # 17/2 = 8.5 is not representable in e4m3; round half of the
        # occurrences up and half down so the error is ~zero-mean.
        s = (g0 + g1 + g2).astype(np.int64)
        idx = np.indices(s.shape).sum(axis=0)  # checkerboard parity
        u[1] = np.where(s == 17, np.where(idx % 2 == 0, 8.0, 9.0), u[1])
    w_sb = u.transpose(4, 1, 0, 5, 3, 2)  # [ki, ocb, u, kh, c, m]
    w_sb = np.ascontiguousarray(w_sb).reshape(128, 2, 12, 2, 128)
    return w_sb.astype(_np_dtype(mode))


def prep_x_wino(x_core, mode="wino8"):
    """x_core [IMGS, 256, 56, 56] int32 -> v_sb [128 ki, IMGS, 4 m, 2 c, 1568]."""
    xq = np.clip(x_core.astype(np.int32), 0, 7)
    e = xq[..., 0::2]  # [n, ch, 56, 28]
    o = xq[..., 1::2]
    e0, e1 = e[..., :JQ], e[..., 1:]
    o0, o1 = o[..., :JQ], o[..., 1:]
    v = np.empty((IMGS, 256, 4, WROWS, WJ), np.float32)
    v[:, :, 0] = e0 - e1
    v[:, :, 1] = o0 + e1
    v[:, :, 2] = e1 - o0
    v[:, :, 3] = o0 - o1
    v = v.reshape(IMGS, 2, 128, 4, VROW)  # [n, c, ki, m, row]
    v_sb = np.ascontiguousarray(v.transpose(2, 0, 3, 1, 4))  # [ki, n, m, c, row]
    return v_sb.astype(_np_dtype(mode))


def prep_weight(weight, mode="fp8dr"):
    """weight [256, 256, 3, 3] OIHW fp32 -> w_sb [128 ki, 2 oc, 9 tap, 2 c, 128 m]."""
    wq = weight.astype(np.int32).astype(np.float32)
    wq = wq.reshape(2, 128, 2, 128, 3, 3)
    w_sb = np.ascontiguousarray(wq.transpose(3, 0, 4, 5, 2, 1))
    w_sb = w_sb.reshape(128, 2, 9, 2, 128)
    return w_sb.astype(_np_dtype(mode))


def prep_x_core(x_core, mode="fp8dr"):
    """x_core [IMGS, 256, 56, 56] int32 -> x_sb [128 ki, 2 c, IMGS, PIXP]."""
    xq = np.clip(x_core.astype(np.int32), 0, 7).astype(np.float32)
    xq = xq.reshape(IMGS, 2, 128, PIX)
    x_sb = np.zeros((128, 2, IMGS, PIXP), np.float32)
    x_sb[:, :, :, :PIX] = xq.transpose(2, 1, 0, 3)
    return x_sb.astype(_np_dtype(mode))


def make_lead(w_sb, v_sb):
    """Pack weights + image-0 V into the dense lead tensor."""
    lead = np.empty((128, 6144 + 4320 + 4752 + 4320), w_sb.dtype)
    lead[:, 0:6144] = w_sb.reshape(128, -1)
    lead[:, 6144:10464] = v_sb[:, 0, :, :, 0:540].reshape(128, -1)
    lead[:, 10464:15216] = v_sb[:, 0, :, :, 486:1080].reshape(128, -1)
    lead[:, 15216:19536] = v_sb[:, 0, :, :, 972:1512].reshape(128, -1)
    return lead


def make_in_maps(x, weight, mode="wino8"):
    if mode in ("wino8", "winob"):
        w_sb = prep_weight_wino(weight, mode)
        maps = []
        for c in range(N_CORES):
            v_sb = prep_x_wino(x[c * IMGS : (c + 1) * IMGS], mode)
            maps.append({
                "lead_sb": make_lead(w_sb, v_sb),
                "v_sb": np.ascontiguousarray(v_sb[:, 1:]),
            })
        return maps
    w_sb = prep_weight(weight, mode)
    return [
        {"x_sb": prep_x_core(x[c * IMGS : (c + 1) * IMGS], mode), "w_sb": w_sb}
        for c in range(N_CORES)
    ]


def kernel(x, weight):
    import time

    from concourse.bass_utils import run_bass_kernel_spmd

    mode = "wino8"
    nc = get_program(mode)
    in_maps = make_in_maps(np.asarray(x), np.asarray(weight), mode)
    last_err = None
    for attempt in range(3):
        try:
            res = run_bass_kernel_spmd(nc, in_maps, list(range(N_CORES)))
            break
        except Exception as e:  # transient NRT_EXEC_UNIT_UNRECOVERABLE flakes
            last_err = e
            time.sleep(2.0)
    else:
        raise last_err
    return np.concatenate(
        [res.results[c]["out"] for c in range(N_CORES)], axis=0
    ).astype(np.float32)
